# revision 1
# baseline (speedup 1.0000x reference)
"""DGP loss kernel for Trainium2, 8 NeuronCores, pure data parallel.

Math (algebraically identical to the reference):
  - The reference extracts overlapping 5x5 patches (stride 1) of the 4x-downsampled
    depth grid and takes a masked lower-median over each 4x4 depth sub-block.
    Sub-block (u,v) of patch (a,b) is exactly the aligned 4x4 depth block at
    feature-grid cell (a+u, b+v), so we compute the median once per cell:
    M[i,j], i<128, j<256.
  - Since normalized disparity n(d) is monotone DECREASING in d, the lower
    median of the n-values equals n(idx-th LARGEST valid d), idx=(k-1)//2.
    We sort raw clamped depths descending (key = -d, invalid -> +BIG) and
    apply n() to the single selected value - no full-tensor reciprocal.
  - seg branch: with sf = F.normalize(seg_feat, dim=C), the patch term is
    ||sf_c - sf_n||^2 = 2 - 2*dot(sf_c, sf_n), so
    loss_term(center,offset) = exp(-|M_c - M_n|) * exp(2*dot - 2)
    summed over centers i in [2,126), j in [2,254) and the 24 offsets
    (du,dv) in [-2,2]^2 minus (0,0); valid_amount = sum (M_c>0)&(M_n>0).
  - per_img = sum(terms)/max(valid_amount,1); loss = mean over images.

Sharding: 8 cores = 2 images x 4 column bands (63 centers each, +2 halo).

Layouts (engine APs must start at partition 0/32/64/96, so row shifts must
live in the free dimension):
  - depth/median work: [i=128 partitions, free] (no shifted operands needed)
  - correlation: sf in strip layout [(s:4 x c:32)=128 partitions,
    (ii:35, j:68 padded) free]; strip s holds feature rows 31s..31s+35; both
    du and dv shifts are free-dim offsets. The row stride is padded to 68 so
    bf16 operand starts keep 4-byte alignment (dv odd handled by a 1-column
    pre-shifted bf16 copy). Channel reduction via PE matmul with a
    block-diagonal ones lhsT -> PSUM [4, 31*64]; an ACT copy (bf16) + DMA
    regroups (s, ii) -> center-row partitions.
  - shifted copies of M (5 row shifts) via SBUF->SBUF DMAs (DMA has no
    partition-start restriction).
"""

from contextlib import ExitStack

import numpy as np

import concourse.bass as bass
import concourse.mybir as mybir
import concourse.tile as tile
from concourse import bass_utils
from concourse.alu_op_type import AluOpType

F32 = mybir.dt.float32
BF16 = mybir.dt.bfloat16
AF = mybir.ActivationFunctionType

EPS_FN = 1e-8
BIG = 3.0e38
MIN_D, MAX_D = 1.0, 100.0
NSCALE = 1.0 / (1.0 / MIN_D - 1.0 / MAX_D)          # 1/0.99
NBIAS = -(1.0 / MAX_D) * NSCALE                      # -0.01/0.99
# n(d) > EPS_FN  <=>  d < DD_THR (n is monotone decreasing)
DD_THR = float(1.0 / (1.0 / MAX_D + EPS_FN / NSCALE))

# Per-core shard geometry (2 images x 4 bands of 63 centers).
JBAND = 63          # centers per band
JCOLS = 67          # feature cols loaded (halo 2 each side)
JP = 68             # padded row stride (even, for bf16 4B alignment)
JW = 64             # padded correlation width (63 centers + 1 junk col)
DCOLS = 4 * JCOLS   # depth cols loaded
NROW = 35           # feature rows per strip (31 centers + 4 halo)
NCEN = 31           # center rows per strip
NP = NCEN * JW      # per-offset dot count (incl junk col) = 1984
NF = NROW * JP      # strip free size = 2380


def _oddeven_merge_sort_layers(n):
    """Batcher odd-even mergesort compare-exchange pairs, grouped by layer."""
    layers = []
    p = 1
    while p < n:
        k = p
        while k >= 1:
            layer = []
            for j in range(k % p, n - k, 2 * k):
                for i in range(0, min(k, n - j - k)):
                    if (i + j) // (p * 2) == (i + j + k) // (p * 2):
                        layer.append((i + j, i + j + k))
            layers.append(layer)
            k //= 2
        p *= 2
    return layers


def _group_runs(pairs):
    """Group CE pairs (a, a+d) of one layer into (a0, step, count, d) runs
    with arithmetic-progression a's so each run is one strided AP op."""
    by_d = {}
    for a, b in pairs:
        by_d.setdefault(b - a, []).append(a)
    runs = []
    for d, alist in sorted(by_d.items()):
        alist = sorted(alist)
        i = 0
        while i < len(alist):
            j = i + 1
            step = None
            while j < len(alist):
                s = alist[j] - alist[j - 1]
                if step is None:
                    step = s
                elif s != step:
                    break
                j += 1
            cnt = j - i
            runs.append((alist[i], step if cnt > 1 else 1, cnt, d))
            i = j
    return runs


def _planes(t, start, step, count):
    """AP over plane dim of a [128, NPLANES, W] tile."""
    if count == 1:
        return t[:, start : start + 1, :]
    return t[:, start : start + (count - 1) * step + 1 : step, :]


def _split_excess_waits(nc, max_waits=1):
    """This container's walrus build rejects instructions carrying more than
    one sem-wait ("Too many sync wait commands"); Tile's scheduler happily
    attaches several. Move the excess onto standalone EventSemaphore waits
    immediately before the instruction on the same engine queue."""
    for f in nc.m.functions:
        for blk in f.blocks:
            new_insts = []
            for inst in blk.instructions:
                si = inst.sync_info
                if si is not None and si.on_wait and len(si.on_wait) > max_waits:
                    waits = list(si.on_wait)
                    excess, keep = waits[:-max_waits], waits[-max_waits:]
                    idx = 0
                    while excess:
                        chunk, excess = excess[:max_waits], excess[max_waits:]
                        new_insts.append(
                            mybir.InstEventSemaphore(
                                name=f"{inst.name}-wsplit{idx}",
                                engine=inst.engine,
                                ins=[],
                                outs=[],
                                sync_info=mybir.SyncInfo(on_wait=chunk, on_update=[]),
                            )
                        )
                        idx += 1
                    si.on_wait = keep
                new_insts.append(inst)
            blk.instructions[:] = new_insts


def _act_rsqrt(nc, out, in_, bias_ap):
    """Raw Rsqrt InstActivation: out = 1/sqrt(in_ + bias). bass's activation()
    refuses Rsqrt on accuracy grounds; our tolerance budget absorbs it and the
    end-to-end rel-err check guards the result."""
    act = nc.scalar
    inputs = [
        act.lower_ap(in_),
        act.lower_ap(bias_ap),
        mybir.ImmediateValue(dtype=mybir.dt.float32, value=1.0),
        mybir.ImmediateValue(dtype=mybir.dt.float32, value=0.0),
    ]
    return act.add_instruction(
        mybir.InstActivation(
            name=nc.get_next_instruction_name(),
            func=AF.Rsqrt,
            ins=inputs,
            outs=[act.lower_ap(out)],
        )
    )


def _build_core_program(split_waits=True):
    nc = bass.Bass("TRN2", target_bir_lowering=False, debug=False)
    dep = nc.dram_tensor("dep", [512, DCOLS], F32, kind="ExternalInput")
    sf = nc.dram_tensor("sf", [32, 128, JCOLS], F32, kind="ExternalInput")
    out = nc.dram_tensor("out", [124, 4], F32, kind="ExternalOutput")

    with tile.TileContext(nc) as tc, ExitStack() as ctx:
        persist = ctx.enter_context(tc.tile_pool(name="persist", bufs=1))
        work = ctx.enter_context(tc.tile_pool(name="work", bufs=1))
        prods = ctx.enter_context(tc.tile_pool(name="prods", bufs=3))

        v = nc.vector
        act = nc.scalar

        # ---------------- depth branch ([i=128, ...] layout) ----------------
        dep_raw = work.tile([128, 4, DCOLS], F32)
        nc.sync.dma_start(
            out=dep_raw, in_=dep.ap().rearrange("(i r) w -> i r w", r=4)
        )
        dcl = work.tile([128, 4, DCOLS], F32)
        v.tensor_scalar(dcl, dep_raw, MIN_D, MAX_D, op0=AluOpType.max, op1=AluOpType.min)
        vld = work.tile([128, 4, DCOLS], F32)
        v.tensor_tensor(vld, dcl, dep_raw, op=AluOpType.is_equal)
        thrm = work.tile([128, 4, DCOLS], F32)
        v.tensor_scalar(thrm, dcl, DD_THR, None, op0=AluOpType.is_lt)
        valid = work.tile([128, 4, DCOLS], F32)
        v.tensor_tensor(valid, vld, thrm, op=AluOpType.mult)

        # valid count k per 4x4 block
        k_t = persist.tile([128, JCOLS], F32)
        v.reduce_sum(
            out=k_t,
            in_=valid.rearrange("p r (j s) -> p j r s", s=4),
            axis=mybir.AxisListType.XY,
        )

        # sort key: valid ? -d : +BIG  (ascending sort = descending depth);
        # built as (-1)*(valid*d) + BIG*(1-valid) to avoid f32 absorption
        dv_t = work.tile([128, 4, DCOLS], F32)
        v.tensor_tensor(dv_t, valid, dcl, op=AluOpType.mult)
        nvb = work.tile([128, 4, DCOLS], F32)
        v.tensor_scalar(nvb, valid, 0.5, BIG, op0=AluOpType.is_lt, op1=AluOpType.mult)
        ndv = work.tile([128, 4, DCOLS], F32)
        v.tensor_scalar(ndv, dv_t, -1.0, None, op0=AluOpType.mult)
        S = persist.tile([128, 16, JP], BF16)
        v.memset(S, BIG)
        v.tensor_tensor(
            out=S[:, :, 0:JCOLS].rearrange("p (r s) j -> p r j s", s=4),
            in0=ndv.rearrange("p r (j s) -> p r j s", s=4),
            in1=nvb.rearrange("p r (j s) -> p r j s", s=4),
            op=AluOpType.add,
        )
        for layer in _oddeven_merge_sort_layers(16):
            for a0, astep, cnt, d in _group_runs(layer):
                lo = _planes(S, a0, astep, cnt)
                hi = _planes(S, a0 + d, astep, cnt)
                tmp = prods.tile([128, cnt, JP], BF16, tag="cetmp")
                v.tensor_tensor(tmp[:, :cnt, :], lo, hi, op=AluOpType.max)
                v.tensor_tensor(lo, lo, hi, op=AluOpType.min)
                v.tensor_copy(hi, tmp[:, :cnt, :])

        # lower-median select: u_m = [k>=2m+1] - [k>=2m+3], m=0..7
        G = work.tile([128, 9, JCOLS], BF16)
        for m in range(9):
            v.tensor_scalar(G[:, m, :], k_t, float(2 * m + 1), None, op0=AluOpType.is_ge)
        u = work.tile([128, 8, JCOLS], BF16)
        v.tensor_tensor(u, G[:, 0:8, :], G[:, 1:9, :], op=AluOpType.subtract)
        sel = work.tile([128, 8, JCOLS], BF16)
        v.tensor_tensor(sel, S[:, 0:8, 0:JCOLS], u, op=AluOpType.mult)
        mdneg = work.tile([128, JCOLS], F32)
        v.reduce_sum(
            out=mdneg, in_=sel.rearrange("p m j -> p j m"), axis=mybir.AxisListType.X
        )
        # med_d = max(-mdneg, 1); M = (1/med_d * NSCALE + NBIAS) * [k>=1]
        med_d = work.tile([128, JCOLS], F32)
        v.tensor_scalar(med_d, mdneg, -1.0, 1.0, op0=AluOpType.mult, op1=AluOpType.max)
        rec_s = work.tile([128, JCOLS], F32)
        v.reciprocal(rec_s, med_d)
        aff_s = work.tile([128, JCOLS], F32)
        act.activation(aff_s, rec_s, AF.Copy, bias=NBIAS, scale=NSCALE)
        # Kpos = [k>=1] is EXACTLY the reference's (median>0) mask, since all
        # valid disparities exceed EPS_FN; keep it in exact f32 (no bf16).
        Kpos = persist.tile([128, JP], F32)
        v.memset(Kpos, 0.0)
        v.tensor_scalar(Kpos[:, 0:JCOLS], k_t, 0.5, None, op0=AluOpType.is_ge)
        M = persist.tile([128, JP], F32)
        v.memset(M, 0.0)
        v.tensor_tensor(M[:, 0:JCOLS], aff_s, Kpos[:, 0:JCOLS], op=AluOpType.mult)

        # shifted copies xx_sh[d][p, j] = xx[row p+d, col j]
        M_sh = []
        K_sh = []
        for d in range(5):
            mt = persist.tile([124, JP], F32, tag=f"M_sh{d}")
            nc.sync.dma_start(out=mt, in_=M[d : d + 124, :])
            M_sh.append(mt)
            kt2 = persist.tile([124, JP], F32, tag=f"K_sh{d}")
            nc.sync.dma_start(out=kt2, in_=Kpos[d : d + 124, :])
            K_sh.append(kt2)

        # ---------------- seg branch (strip layout) ----------------
        sf_strip = persist.tile([128, NROW, JP], F32)
        v.memset(sf_strip[:, :, JCOLS:JP], 0.0)
        for s in range(4):
            nc.sync.dma_start(
                out=sf_strip[32 * s : 32 * (s + 1), :, 0:JCOLS],
                in_=sf.ap()[:, NCEN * s : NCEN * s + NROW, :],
            )

        # block-diagonal ones (f32 for the f32 nrm2 reduce, bf16 for prods)
        ones4f = persist.tile([128, 4], F32)
        v.memset(ones4f, 0.0)
        ones4b = persist.tile([128, 4], BF16)
        v.memset(ones4b, 0.0)
        for s in range(4):
            v.memset(ones4f[32 * s : 32 * (s + 1), s : s + 1], 1.0)
            v.memset(ones4b[32 * s : 32 * (s + 1), s : s + 1], 1.0)
        eps_b = persist.tile([4, 1], F32)
        v.memset(eps_b, 1e-24)

        f2 = work.tile([128, NROW, JP], F32)
        v.tensor_tensor(f2, sf_strip, sf_strip, op=AluOpType.mult)
        rinv = work.tile([4, NF], F32)
        with tc.tile_pool(name="psnrm", bufs=1, space="PSUM") as psnrm:
            nrm2_ps = psnrm.tile([4, NF], F32, tag="nrm2")
            for c0 in range(0, NF, 512):
                cw = min(512, NF - c0)
                nc.tensor.matmul(
                    nrm2_ps[:, c0 : c0 + cw],
                    ones4f,
                    f2.rearrange("p a b -> p (a b)")[:, c0 : c0 + cw],
                )
            # rinv = 1/sqrt(nrm2 + 1e-24)  (pad cols are all-zero)
            _act_rsqrt(nc, rinv, nrm2_ps, eps_b)
        # broadcast rinv [4, NF] -> [128, NF] (strip row s -> partitions 32s..):
        # DMA with a step-0 free dim on the source (partition step-0 is
        # rejected by the DMA lowering, free-dim replication is fine)
        rinv_rep = work.tile([128, NF], F32)
        for s in range(4):
            nc.sync.dma_start(
                out=rinv_rep[32 * s : 32 * (s + 1), :],
                in_=rinv[s : s + 1, :].unsqueeze(1).broadcast_to((1, 32, NF)),
            )
        sfn = persist.tile([128, NROW, JP], F32)
        v.tensor_tensor(
            sfn.rearrange("p a b -> p (a b)"),
            sf_strip.rearrange("p a b -> p (a b)"),
            rinv_rep,
            op=AluOpType.mult,
        )
        sfb = persist.tile([128, NROW, JP], BF16)
        v.tensor_copy(sfb, sfn)
        # 1-col-left-shifted copy for odd dv offsets (keeps bf16 4B alignment)
        sfb1 = persist.tile([128, NROW, JP], BF16)
        v.memset(sfb1[:, :, JP - 1 : JP], 0.0)
        v.tensor_copy(sfb1[:, :, 0 : JP - 1], sfn[:, :, 1:JP])

        # ---------------- 25-offset correlation ----------------
        psum = ctx.enter_context(tc.tile_pool(name="psum", bufs=2, space="PSUM"))
        dots_b = persist.tile([124, 25, JW], BF16)
        dda = persist.tile([124, 25, JW], F32)
        vm = persist.tile([124, 25, JW], F32)

        cen = sfb[:, 2 : 2 + NCEN, 2 : 2 + JW]
        for o in range(25):
            du, dv = o // 5, o % 5
            src, dvv = (sfb1, dv - 1) if dv % 2 == 1 else (sfb, dv)
            par = src[:, du : du + NCEN, dvv : dvv + JW]
            prod = prods.tile([128, NCEN, JW], BF16, tag="prod")
            v.tensor_tensor(prod, cen, par, op=AluOpType.mult)
            dps = psum.tile([4, NP], F32, tag="dps")
            for c0 in range(0, NP, 512):
                cw = min(512, NP - c0)
                nc.tensor.matmul(
                    dps[:, c0 : c0 + cw],
                    ones4b,
                    prod.rearrange("p a b -> p (a b)")[:, c0 : c0 + cw],
                )
            # regroup PSUM [s, (ii, j)] -> dots[p = s*31+ii, o, j]
            dsb = prods.tile([4, NP], BF16, tag="dsb")
            act.activation(dsb, dps, AF.Copy)
            nc.sync.dma_start(
                out=dots_b[:, o, :],
                in_=dsb.rearrange("s (a b) -> s a b", a=NCEN),
            )

            Mc = M_sh[2][:, 2 : 2 + JW]
            Mn = M_sh[du][:, dv : dv + JW]
            v.tensor_tensor(dda[:, o, :], Mc, Mn, op=AluOpType.subtract)
            v.tensor_tensor(
                vm[:, o, :],
                K_sh[2][:, 2 : 2 + JW],
                K_sh[du][:, dv : dv + JW],
                op=AluOpType.mult,
            )

        act.activation(dda, dda, AF.Abs)

        dots_f = work.tile([124, 25, JW], F32)
        v.tensor_copy(dots_f, dots_b)
        xt = work.tile([124, 25, JW], F32)
        v.scalar_tensor_tensor(
            xt, dots_f, 2.0, dda, op0=AluOpType.mult, op1=AluOpType.subtract
        )
        bias_m2 = persist.tile([124, 1], F32)
        v.memset(bias_m2, -2.0)
        terms = work.tile([124, 25, JW], F32)
        act.activation(terms, xt, AF.Exp, bias=bias_m2, scale=1.0)

        numden = work.tile([124, 4], F32)
        v.reduce_sum(
            out=numden[:, 0:1], in_=terms[:, 0:12, 0:JBAND], axis=mybir.AxisListType.XY
        )
        v.reduce_sum(
            out=numden[:, 1:2], in_=terms[:, 13:25, 0:JBAND], axis=mybir.AxisListType.XY
        )
        v.reduce_sum(
            out=numden[:, 2:3], in_=vm[:, 0:12, 0:JBAND], axis=mybir.AxisListType.XY
        )
        v.reduce_sum(
            out=numden[:, 3:4], in_=vm[:, 13:25, 0:JBAND], axis=mybir.AxisListType.XY
        )
        nc.sync.dma_start(out=out.ap(), in_=numden)

    if split_waits:
        _split_excess_waits(nc)
    return nc


_NC_CACHE = []


def kernel(seg_feat: np.ndarray, dep_true: np.ndarray) -> np.ndarray:
    seg_feat = np.ascontiguousarray(seg_feat, dtype=np.float32)
    dep_true = np.ascontiguousarray(dep_true, dtype=np.float32)

    if not _NC_CACHE:
        _NC_CACHE.append(_build_core_program())
    nc = _NC_CACHE[0]

    in_maps = []
    for core in range(8):
        img, band = core // 4, core % 4
        j0 = JBAND * band
        in_maps.append(
            {
                "dep": np.ascontiguousarray(dep_true[img, :, 4 * j0 : 4 * j0 + DCOLS]),
                "sf": np.ascontiguousarray(seg_feat[img, :, :, j0 : j0 + JCOLS]),
            }
        )

    res = bass_utils.run_bass_kernel_spmd(nc, in_maps, core_ids=list(range(8)))
    parts = [r["out"].astype(np.float64) for r in res.results]

    loss = 0.0
    for img in range(2):
        num = sum(parts[img * 4 + b][:, 0:2].sum() for b in range(4))
        den = sum(parts[img * 4 + b][:, 2:4].sum() for b in range(4))
        loss += num / max(den, 1.0)
    return np.float32(loss / 2.0)



# revision 8
# speedup vs baseline: 1.3187x; 1.3187x over previous
"""DGP loss kernel for Trainium2, 8 NeuronCores, pure data parallel.

Math (algebraically identical to the reference):
  - Per-feature-cell masked lower-median M[i,j] over the aligned 4x4 depth
    block (same reduction as the reference's patch sub-blocks; see baseline
    notes). Valid <=> d in [MIN_D, DD_THR). Median selected from a bitonic
    sort of the 16 keys (-d valid / +BIG invalid); k-dependent index picked
    with indicators [k>=t] <=> sorted_key[t-1] < 0 (no separate k count).
  - seg term: ||sf_c - sf_n||^2 = 2 - 2*dot  =>  per-pair term
    exp(2*dot-2) * exp(-|M_c - M_n|); denominator pairs (M_c>0)&(M_n>0).
  - SYMMETRY: term(x, x+o) == term(x+o, x), so only the 12 lex-positive
    offsets are computed; a host-built weight tensor w[p,o,c] in {0,1,2}
    supplies the pair multiplicities (2 in the interior, 1 on the global
    border strips, 0 on junk/padding lanes), which also handles band
    ownership so no pair is counted twice across cores.

Sharding: 8 cores = 2 images x 4 column bands (63 anchor cols each, +2 halo:
67-col loaded block). Anchor partitions p=0..127 <-> feature rows r=p-1
(r=-1,126 are w=0 junk).

Engine plan (from baseline trace: DVE was 92us busy, scalar 53us on PSUM
copies, 45us of pathological broadcast DMA):
  - normalization broadcast 4->128 partitions via PE matmul (ones lhsT),
    not DMA.
  - dots: DVE bf16 products [128,32,68] -> PE ones-reduce into PSUM
    quadrants [0:4]/[32:36] (double buffer in 5 banks) -> scalar ACT
    evacuates with fused exp(2*dots-2) -> bf16 -> DMA regroup to anchor
    layout.
  - sort: ping-pong 2-op compare-exchange layers (min/max into alternate
    buffer + copies of untouched planes).
"""

from contextlib import ExitStack

import numpy as np
import ml_dtypes

import concourse.bass as bass
import concourse.mybir as mybir
import concourse.tile as tile
from concourse import bass_utils
from concourse.alu_op_type import AluOpType

F32 = mybir.dt.float32
BF16 = mybir.dt.bfloat16
AF = mybir.ActivationFunctionType
AX = mybir.AxisListType

EPS_FN = 1e-8
BIG = 3.0e38
MIN_D, MAX_D = 1.0, 100.0
NSCALE = 1.0 / (1.0 / MIN_D - 1.0 / MAX_D)          # 1/0.99
NBIAS = -(1.0 / MAX_D) * NSCALE                      # -0.01/0.99
# n(d) > EPS_FN  <=>  d < DD_THR (n is monotone decreasing)
DD_THR = float(1.0 / (1.0 / MAX_D + EPS_FN / NSCALE))

# Geometry (per core).
JBAND = 63          # anchor cols owned per band
JCOLS = 67          # feature cols loaded (63 + 2 halo each side)
JP = 72             # padded row stride (data at cols 2..68, zero pads around)
JW = 68             # correlation width (67 anchors + 1 junk col, even)
DCOLS = 4 * JCOLS   # depth cols loaded
NROW = 36           # feature rows per strip (32 anchors + halo)
NCEN = 32           # anchor rows per strip
NP = NCEN * JW      # per-offset dot count = 2176
NF = NROW * JP      # strip free size = 2592

# 12 lex-positive offsets (du, dv): pairs (x, x + o)
OFFS = [(0, 1), (0, 2),
        (1, -2), (1, -1), (1, 0), (1, 1), (1, 2),
        (2, -2), (2, -1), (2, 0), (2, 1), (2, 2)]
NO = len(OFFS)


def _oddeven_merge_sort_layers(n):
    """Batcher odd-even mergesort compare-exchange pairs, grouped by layer."""
    layers = []
    p = 1
    while p < n:
        k = p
        while k >= 1:
            layer = []
            for j in range(k % p, n - k, 2 * k):
                for i in range(0, min(k, n - j - k)):
                    if (i + j) // (p * 2) == (i + j + k) // (p * 2):
                        layer.append((i + j, i + j + k))
            layers.append(layer)
            k //= 2
        p *= 2
    return layers


def _group_runs(pairs):
    """Group CE pairs (a, a+d) of one layer into (a0, step, count, d) runs
    with arithmetic-progression a's so each run is one strided AP op."""
    by_d = {}
    for a, b in pairs:
        by_d.setdefault(b - a, []).append(a)
    runs = []
    for d, alist in sorted(by_d.items()):
        alist = sorted(alist)
        i = 0
        while i < len(alist):
            j = i + 1
            step = None
            while j < len(alist):
                s = alist[j] - alist[j - 1]
                if step is None:
                    step = s
                elif s != step:
                    break
                j += 1
            cnt = j - i
            runs.append((alist[i], step if cnt > 1 else 1, cnt, d))
            i = j
    return runs


def _ap_runs(idxs):
    """Group a sorted index list into (start, step, count) arithmetic runs."""
    runs = []
    i = 0
    while i < len(idxs):
        j = i + 1
        step = None
        while j < len(idxs):
            s = idxs[j] - idxs[j - 1]
            if step is None:
                step = s
            elif s != step:
                break
            j += 1
        cnt = j - i
        runs.append((idxs[i], step if cnt > 1 else 1, cnt))
        i = j
    return runs


def _planes(t, start, step, count):
    """AP over plane dim of a [128, NPLANES, W] tile."""
    if count == 1:
        return t[:, start : start + 1, :]
    return t[:, start : start + (count - 1) * step + 1 : step, :]


def _split_excess_waits(nc, max_waits=1):
    """This container's walrus build rejects instructions carrying more than
    one sem-wait; move the excess onto standalone EventSemaphore waits."""
    for f in nc.m.functions:
        for blk in f.blocks:
            new_insts = []
            for inst in blk.instructions:
                si = inst.sync_info
                if si is not None and si.on_wait and len(si.on_wait) > max_waits:
                    waits = list(si.on_wait)
                    excess, keep = waits[:-max_waits], waits[-max_waits:]
                    idx = 0
                    while excess:
                        chunk, excess = excess[:max_waits], excess[max_waits:]
                        new_insts.append(
                            mybir.InstEventSemaphore(
                                name=f"{inst.name}-wsplit{idx}",
                                engine=inst.engine,
                                ins=[],
                                outs=[],
                                sync_info=mybir.SyncInfo(on_wait=chunk, on_update=[]),
                            )
                        )
                        idx += 1
                    si.on_wait = keep
                new_insts.append(inst)
            blk.instructions[:] = new_insts


def _act_rsqrt(nc, out, in_, bias_ap):
    """Raw Rsqrt InstActivation: out = 1/sqrt(in_ + bias)."""
    act = nc.scalar
    inputs = [
        act.lower_ap(in_),
        act.lower_ap(bias_ap),
        mybir.ImmediateValue(dtype=mybir.dt.float32, value=1.0),
        mybir.ImmediateValue(dtype=mybir.dt.float32, value=0.0),
    ]
    return act.add_instruction(
        mybir.InstActivation(
            name=nc.get_next_instruction_name(),
            func=AF.Rsqrt,
            ins=inputs,
            outs=[act.lower_ap(out)],
        )
    )


def _build_core_program(split_waits=True):
    nc = bass.Bass("TRN2", target_bir_lowering=False, debug=False)
    dep = nc.dram_tensor("dep", [512, DCOLS], F32, kind="ExternalInput")
    sf = nc.dram_tensor("sf", [32, 128, JCOLS], F32, kind="ExternalInput")
    wt = nc.dram_tensor("w", [128, NO, JW], BF16, kind="ExternalInput")
    obc = nc.dram_tensor("obc", [4, 128], F32, kind="ExternalInput")
    out = nc.dram_tensor("out", [128, 2], F32, kind="ExternalOutput")

    with tile.TileContext(nc) as tc, ExitStack() as ctx:
        persist = ctx.enter_context(tc.tile_pool(name="persist", bufs=1))
        work = ctx.enter_context(tc.tile_pool(name="work", bufs=1))
        prods = ctx.enter_context(tc.tile_pool(name="prods", bufs=3))

        v = nc.vector
        act = nc.scalar

        # ---------------- input DMAs ----------------
        dep_raw = work.tile([128, 4, DCOLS], F32)
        nc.sync.dma_start(out=dep_raw, in_=dep.ap().rearrange("(i r) w -> i r w", r=4))

        sf_strip = persist.tile([128, NROW, JP], F32)
        # zero pads: cols 0:2 and 69:72 everywhere, plus missing halo rows
        v.memset(sf_strip[:, :, 0:2], 0.0)
        v.memset(sf_strip[:, :, JP - 3 : JP], 0.0)
        v.memset(sf_strip[0:32, 0:3, :], 0.0)
        v.memset(sf_strip[96:128, NROW - 1 : NROW, :], 0.0)
        # strip s holds feature rows 32s-3 .. 32s+32 at planes 0..35
        strip_rows = [(0, 33, 3), (29, 65, 0), (61, 97, 0), (93, 128, 0)]
        for s, (r0, r1, pl0) in enumerate(strip_rows):
            nc.sync.dma_start(
                out=sf_strip[32 * s : 32 * (s + 1), pl0 : pl0 + (r1 - r0), 2 : 2 + JCOLS],
                in_=sf.ap()[:, r0:r1, :],
            )

        w_t = persist.tile([128, NO, JW], BF16)
        nc.sync.dma_start(out=w_t, in_=wt.ap())

        # ---------------- seg normalization ----------------
        # block-diagonal ones for strip reduction (bf16) and f32 for nrm2
        ones4f = persist.tile([128, 4], F32)
        v.memset(ones4f, 0.0)
        ones4b = persist.tile([128, 4], BF16)
        v.memset(ones4b, 0.0)
        for s in range(4):
            v.memset(ones4f[32 * s : 32 * (s + 1), s : s + 1], 1.0)
            v.memset(ones4b[32 * s : 32 * (s + 1), s : s + 1], 1.0)
        # broadcast lhsT: [4, 128], row s -> partitions 32s..32s+31 (host input:
        # engine memsets cannot start at partitions 1..3)
        ones_bc = persist.tile([4, 128], F32)
        nc.sync.dma_start(out=ones_bc, in_=obc.ap())
        eps_b = persist.tile([4, 1], F32)
        v.memset(eps_b, 1e-24)
        bias_m2 = persist.tile([4, 1], F32)
        v.memset(bias_m2, -2.0)
        bias_nb = persist.tile([128, 1], F32)
        v.memset(bias_nb, NBIAS)
        bias_z = persist.tile([128, 1], F32)
        v.memset(bias_z, 0.0)

        f2 = work.tile([128, NROW, JP], F32)
        act.activation(f2, sf_strip, AF.Square)
        rinv = work.tile([4, NF], F32)
        with tc.tile_pool(name="psnrm", bufs=1, space="PSUM") as psnrm:
            nrm2_ps = psnrm.tile([4, NF], F32, tag="nrm2")
            for c0 in range(0, NF, 512):
                cw = min(512, NF - c0)
                nc.tensor.matmul(
                    nrm2_ps[:, c0 : c0 + cw],
                    ones4f,
                    f2.rearrange("p a b -> p (a b)")[:, c0 : c0 + cw],
                )
            _act_rsqrt(nc, rinv, nrm2_ps, eps_b)

        sfb = persist.tile([128, NROW, JP], BF16)
        with tc.tile_pool(name="psbc", bufs=1, space="PSUM") as psbc:
            bc_ps = psbc.tile([128, NF], F32, tag="bcast")
            for c0 in range(0, NF, 512):
                cw = min(512, NF - c0)
                nc.tensor.matmul(
                    bc_ps[:, c0 : c0 + cw], ones_bc, rinv[:, c0 : c0 + cw]
                )
            v.tensor_tensor(
                sfb.rearrange("p a b -> p (a b)"),
                sf_strip.rearrange("p a b -> p (a b)"),
                bc_ps,
                op=AluOpType.mult,
            )
        # 1-col-left-shifted copy for odd dv offsets (keeps bf16 4B alignment)
        sfb1 = persist.tile([128, NROW, JP], BF16)
        v.memset(sfb1[:, :, JP - 1 : JP], 0.0)
        nc.sync.dma_start(
            out=sfb1.rearrange("p a b -> p (a b)")[:, 0 : NF - 1],
            in_=sfb.rearrange("p a b -> p (a b)")[:, 1:NF],
        )

        # ---------------- depth branch: valid mask + sort keys ----------------
        # depn = -d (negated cast); valid <=> depn <= -1 AND depn > -THR
        depn = work.tile([128, 4, DCOLS], BF16)
        v.tensor_scalar(depn, dep_raw, -1.0, None, op0=AluOpType.mult)
        le1 = work.tile([128, 4, DCOLS], BF16)
        v.tensor_scalar(le1, depn, -MIN_D, None, op0=AluOpType.is_le)
        gtT = work.tile([128, 4, DCOLS], BF16)
        v.tensor_scalar(gtT, depn, -DD_THR, None, op0=AluOpType.is_gt)
        valid = work.tile([128, 4, DCOLS], BF16)
        v.tensor_tensor(valid, le1, gtT, op=AluOpType.mult)
        dv_t = work.tile([128, 4, DCOLS], BF16)
        v.tensor_tensor(dv_t, valid, depn, op=AluOpType.mult)
        nvb = work.tile([128, 4, DCOLS], BF16)
        v.tensor_scalar(nvb, valid, 0.5, BIG, op0=AluOpType.is_lt, op1=AluOpType.mult)

        # sort keys: valid ? -d : +BIG; planes (r, s) of the 4x4 block
        SA = persist.tile([128, 16, JW], BF16, tag="SA")
        SB = persist.tile([128, 16, JW], BF16, tag="SB")
        v.memset(SA, BIG)
        v.tensor_tensor(
            out=SA[:, :, 0:JCOLS].rearrange("p (r s) j -> p r j s", s=4),
            in0=dv_t.rearrange("p r (j s) -> p r j s", s=4),
            in1=nvb.rearrange("p r (j s) -> p r j s", s=4),
            op=AluOpType.add,
        )

        # ping-pong 2-op compare-exchange sort
        bufs = [SA, SB]
        layers = _oddeven_merge_sort_layers(16)
        for li, layer in enumerate(layers):
            src, dst = bufs[li % 2], bufs[(li + 1) % 2]
            touched = set()
            for a, b in layer:
                touched.add(a)
                touched.add(b)
            for a0, astep, cnt, d in _group_runs(layer):
                lo_s = _planes(src, a0, astep, cnt)
                hi_s = _planes(src, a0 + d, astep, cnt)
                v.tensor_tensor(_planes(dst, a0, astep, cnt), lo_s, hi_s, op=AluOpType.min)
                v.tensor_tensor(
                    _planes(dst, a0 + d, astep, cnt), lo_s, hi_s, op=AluOpType.max
                )
            untouched = sorted(set(range(16)) - touched)
            for u0, ustep, ucnt in _ap_runs(untouched):
                v.tensor_copy(_planes(dst, u0, ustep, ucnt), _planes(src, u0, ustep, ucnt))
        S = bufs[len(layers) % 2]

        # lower-median select: u_m = [k>=2m+1] - [k>=2m+3]; [k>=t] <=> S[t-1]<0
        G = work.tile([128, 8, JW], BF16)
        v.tensor_scalar(G, S[:, 0:16:2, :], 0.0, None, op0=AluOpType.is_lt)
        u = work.tile([128, 8, JW], BF16)
        v.tensor_tensor(u[:, 0:7, :], G[:, 0:7, :], G[:, 1:8, :], op=AluOpType.subtract)
        v.tensor_copy(u[:, 7:8, :], G[:, 7:8, :])
        sel = work.tile([128, 8, JW], BF16)
        v.tensor_tensor(sel, S[:, 0:8, :], u, op=AluOpType.mult)
        mdneg = work.tile([128, JW], F32)
        v.reduce_sum(out=mdneg, in_=sel.rearrange("p m j -> p j m"), axis=AX.X)
        med_d = work.tile([128, JW], F32)
        v.tensor_scalar(med_d, mdneg, -1.0, 1.0, op0=AluOpType.mult, op1=AluOpType.max)
        rec_s = work.tile([128, JW], F32)
        v.reciprocal(rec_s, med_d)
        aff_s = work.tile([128, JW], F32)
        act.activation(aff_s, rec_s, AF.Copy, bias=NBIAS, scale=NSCALE)

        Kpos = persist.tile([128, JP], F32)
        v.memset(Kpos, 0.0)
        v.tensor_scalar(Kpos[:, 2 : 2 + JW], S[:, 0, :], 0.0, None, op0=AluOpType.is_lt)
        Kb = persist.tile([128, JP], BF16)
        v.tensor_copy(Kb, Kpos)
        M = persist.tile([128, JP], F32)
        v.memset(M, 0.0)
        v.tensor_tensor(M[:, 2 : 2 + JW], aff_s, Kpos[:, 2 : 2 + JW], op=AluOpType.mult)

        # row-shifted copies: Xm1[p] = X[p-1], Xp1[p] = X[p+1]
        M_m1 = persist.tile([128, JP], F32, tag="M_m1")
        v.memset(M_m1, 0.0)
        nc.sync.dma_start(out=M_m1[1:128, :], in_=M[0:127, :])
        M_p1 = persist.tile([128, JP], F32, tag="M_p1")
        v.memset(M_p1, 0.0)
        nc.sync.dma_start(out=M_p1[0:127, :], in_=M[1:128, :])
        K_m1 = persist.tile([128, JP], BF16, tag="K_m1")
        v.memset(K_m1, 0.0)
        nc.sync.dma_start(out=K_m1[1:128, :], in_=Kb[0:127, :])
        K_p1 = persist.tile([128, JP], BF16, tag="K_p1")
        v.memset(K_p1, 0.0)
        nc.sync.dma_start(out=K_p1[0:127, :], in_=Kb[1:128, :])
        M_by_du = [M_m1, M, M_p1]
        K_by_du = [K_m1, Kb, K_p1]

        # ---------------- 12-offset correlation ----------------
        dots_rg = persist.tile([128, NO, JW], BF16)   # exp(2*dot - 2), anchor layout
        dda = persist.tile([128, NO, JW], F32)
        vm = persist.tile([128, NO, JW], BF16)

        psum = ctx.enter_context(tc.tile_pool(name="psum", bufs=1, space="PSUM"))
        dps = psum.tile([36, NP], F32, tag="dps")

        cen = sfb[:, 2 : 2 + NCEN, 2 : 2 + JW]
        for o, (du, dv) in enumerate(OFFS):
            if dv % 2 == 0:
                par = sfb[:, 2 + du : 2 + du + NCEN, 2 + dv : 2 + dv + JW]
            else:
                par = sfb1[:, 2 + du : 2 + du + NCEN, 1 + dv : 1 + dv + JW]
            prod = prods.tile([128, NCEN, JW], BF16, tag="prod")
            v.tensor_tensor(prod, cen, par, op=AluOpType.mult)
            q = 32 * (o % 2)
            for c0 in range(0, NP, 512):
                cw = min(512, NP - c0)
                nc.tensor.matmul(
                    dps[q : q + 4, c0 : c0 + cw],
                    ones4b,
                    prod.rearrange("p a b -> p (a b)")[:, c0 : c0 + cw],
                )
            # evacuate with fused exp(2*dots - 2) -> bf16
            dsb = prods.tile([4, NP], BF16, tag="dsb")
            act.activation(dsb, dps[q : q + 4, :], AF.Exp, bias=bias_m2, scale=2.0)
            nc.sync.dma_start(
                out=dots_rg[:, o, :], in_=dsb.rearrange("s (a c) -> s a c", a=NCEN)
            )

            Mc = M_m1[:, 2 : 2 + JW]
            Mn = M_by_du[du][:, 2 + dv : 2 + dv + JW]
            v.tensor_tensor(dda[:, o, :], Mc, Mn, op=AluOpType.subtract)
            v.tensor_tensor(
                vm[:, o, :],
                K_m1[:, 2 : 2 + JW],
                K_by_du[du][:, 2 + dv : 2 + dv + JW],
                op=AluOpType.mult,
            )

        # ---------------- combine ----------------
        act.activation(dda, dda, AF.Abs)
        expdda = work.tile([128, NO, JW], BF16)
        act.activation(expdda, dda, AF.Exp, bias=bias_z, scale=-1.0)
        wexp = work.tile([128, NO, JW], BF16)
        v.tensor_tensor(wexp, expdda, w_t, op=AluOpType.mult)
        terms = work.tile([128, NO, JW], BF16)
        v.tensor_tensor(terms, dots_rg, wexp, op=AluOpType.mult)
        vmw = work.tile([128, NO, JW], BF16)
        v.tensor_tensor(vmw, vm, w_t, op=AluOpType.mult)

        numden = work.tile([128, 2], F32)
        v.reduce_sum(out=numden[:, 0:1], in_=terms, axis=AX.XY)
        v.reduce_sum(out=numden[:, 1:2], in_=vmw, axis=AX.XY)
        nc.sync.dma_start(out=out.ap(), in_=numden)

    if split_waits:
        _split_excess_waits(nc)
    return nc


def _build_weights(band):
    """w[p, o, c] in {0,1,2}: pair multiplicity for anchor (row r=p-1,
    feature col j=63*band+c), offset o -- [x in C]*[col in B] +
    [x+o in C]*[col+dv in B]."""
    p = np.arange(128)[:, None, None]
    r = p - 1
    c = np.arange(JW)[None, None, :]
    j = 63 * band + c
    du = np.array([o[0] for o in OFFS])[None, :, None]
    dv = np.array([o[1] for o in OFFS])[None, :, None]
    j0 = 63 * band + 2
    w1 = (
        (r >= 2) & (r <= 125) & (j >= 2) & (j <= 253) & (j >= j0) & (j <= j0 + 62)
    ).astype(np.float32)
    w2 = (
        (r + du >= 2) & (r + du <= 125)
        & (j + dv >= 2) & (j + dv <= 253)
        & (j + dv >= j0) & (j + dv <= j0 + 62)
    ).astype(np.float32)
    return (w1 + w2).astype(ml_dtypes.bfloat16)


_NC_CACHE = []
_W_CACHE = {}
_OBC = np.zeros((4, 128), dtype=np.float32)
for _s in range(4):
    _OBC[_s, 32 * _s : 32 * (_s + 1)] = 1.0


def make_in_maps(seg_feat, dep_true):
    in_maps = []
    for core in range(8):
        img, band = core // 4, core % 4
        j0 = JBAND * band
        if band not in _W_CACHE:
            _W_CACHE[band] = _build_weights(band)
        in_maps.append(
            {
                "dep": np.ascontiguousarray(dep_true[img, :, 4 * j0 : 4 * j0 + DCOLS]),
                "sf": np.ascontiguousarray(seg_feat[img, :, :, j0 : j0 + JCOLS]),
                "w": _W_CACHE[band],
                "obc": _OBC,
            }
        )
    return in_maps


def kernel(seg_feat: np.ndarray, dep_true: np.ndarray) -> np.ndarray:
    seg_feat = np.ascontiguousarray(seg_feat, dtype=np.float32)
    dep_true = np.ascontiguousarray(dep_true, dtype=np.float32)

    if not _NC_CACHE:
        _NC_CACHE.append(_build_core_program())
    nc = _NC_CACHE[0]

    in_maps = make_in_maps(seg_feat, dep_true)
    res = bass_utils.run_bass_kernel_spmd(nc, in_maps, core_ids=list(range(8)))
    parts = [r["out"].astype(np.float64) for r in res.results]

    loss = 0.0
    for img in range(2):
        num = sum(parts[img * 4 + b][:, 0].sum() for b in range(4))
        den = sum(parts[img * 4 + b][:, 1].sum() for b in range(4))
        loss += num / max(den, 1.0)
    return np.float32(loss / 2.0)
